# revision 19
# baseline (speedup 1.0000x reference)
"""Complex-valued fully-connected layer on 8 TRN2 NeuronCores.

Math (per reference):
    out_re = x_re @ w_re^T - x_im @ w_im^T
    out_im = x_re @ w_im^T + x_im @ w_re^T        -> stack([out_re, out_im])
with x_*: [8192, 2048] f32, w_*: [2048, 2048] f32.

Strategy:
  - Shard 8 cores = 2 batch-halves (4096 rows) x 4 out-feature quarters (512).
    Outputs are disjoint -> no collectives.
  - Karatsuba (3 real GEMMs instead of 4):
        t_rr = x_re @ w_re^T ; t_ii = x_im @ w_im^T ; t_ss = (x_re+x_im)@(w_re+w_im)^T
        out_re = t_rr - t_ii ; out_im = t_ss - t_rr - t_ii
  - float32r matmuls (full-rate PE at free-dim 512), fp32 PSUM accumulate.
  - Host pre-packs operands into DMA-friendly layouts (every descriptor is a
    multi-KiB contiguous run per partition):
      x: [BT, 128p, KT, 128b]  (tile DMA = 128 descriptors x 8 KiB; a 2D
         [in, batch] layout gives 512 B descriptors, which rate-limits the
         HWDGE to ~139 GB/s and starves the PE)
      w: [128p, KT, 512o]      (half-tensor DMA = 128 descriptors x 16 KiB)
  - x tile buffers are 3 deep so the x load latency is NOT inside the
    per-tile critical loop (2-deep made steady state ~14.5 us/tile instead
    of the PE-bound ~10.9 us/tile).
  - Weights go as 4 half-tensor DMAs on the ACT ring (re_lo, im_lo, re_hi,
    im_hi) with threshold waits so tile-0 matmuls start while the tail is
    still streaming. Only SP/ACT HWDGE rings are used for data: the
    GpSimd(Pool) software queue's completion increments can fire before
    its data is visible to another engine (measured: stale w_im baked into
    w_s), so it is not used.
  - Raw bass with explicit semaphores. HWDGE executes FIFO per issuing
    engine; each dma_start's completion adds +16 (one +1 per DMA engine),
    so cumulative thresholds are sound.
"""

import numpy as np

import concourse.bass as bass
from concourse import mybir
from concourse.bass_utils import run_bass_kernel_spmd

BATCH, IN_F, OUT_F = 8192, 2048, 2048
N_CORES = 8
B_SHARDS, O_SHARDS = 2, 4
B_SH = BATCH // B_SHARDS          # 4096 batch rows per core
O_SH = OUT_F // O_SHARDS          # 512 out features per core
KT = IN_F // 128                  # 16 contraction tiles
BT = B_SH // 128                  # 32 batch tiles per core
XD = 3                            # x tile buffer depth
KH = KT // 2                      # weight half split (k 0..7 / 8..15)

F32 = mybir.dt.float32
F32R = mybir.dt.float32r


def build_nc() -> bass.Bass:
    nc = bass.Bass("TRN2", target_bir_lowering=False, debug=False)

    xt_re = nc.dram_tensor("xt_re", [BT, 128, KT, 128], F32R, kind="ExternalInput")
    xt_im = nc.dram_tensor("xt_im", [BT, 128, KT, 128], F32R, kind="ExternalInput")
    wt_re = nc.dram_tensor("wt_re", [128, KT, O_SH], F32R, kind="ExternalInput")
    wt_im = nc.dram_tensor("wt_im", [128, KT, O_SH], F32R, kind="ExternalInput")
    out_d = nc.dram_tensor("out", [2, B_SH, O_SH], F32, kind="ExternalOutput")

    # SBUF: weights resident ([p, k, o]); x XD-deep per b-tile
    # ([p, buf, k, b]); output staging double-buffered ([p, buf, o]).
    w_re_sb = nc.alloc_sbuf_tensor("w_re_sb", [128, KT, O_SH], F32R)
    w_im_sb = nc.alloc_sbuf_tensor("w_im_sb", [128, KT, O_SH], F32R)
    w_s_sb = nc.alloc_sbuf_tensor("w_s_sb", [128, KT, O_SH], F32R)
    x_re_sb = nc.alloc_sbuf_tensor("x_re_sb", [128, XD, KT, 128], F32R)
    x_im_sb = nc.alloc_sbuf_tensor("x_im_sb", [128, XD, KT, 128], F32R)
    x_s_sb = nc.alloc_sbuf_tensor("x_s_sb", [128, XD, KT, 128], F32R)
    o_re_sb = nc.alloc_sbuf_tensor("o_re_sb", [128, 2, O_SH], F32)
    o_im_sb = nc.alloc_sbuf_tensor("o_im_sb", [128, 2, O_SH], F32)
    r_sb = nc.alloc_sbuf_tensor("r_sb", [128, 2, O_SH], F32)  # t_rr staging

    p_rr = [nc.alloc_psum_tensor(f"p_rr{b}", [128, O_SH], F32) for b in range(2)]
    p_ii = [nc.alloc_psum_tensor(f"p_ii{b}", [128, O_SH], F32) for b in range(2)]
    p_ss = [nc.alloc_psum_tensor(f"p_ss{b}", [128, O_SH], F32) for b in range(2)]

    with (
        nc.Block() as block,
        nc.semaphore("dma_x") as dma_x,      # SP ring: x_re/x_im tile loads
        nc.semaphore("dma_w") as dma_w,      # ACT ring: 4 weight half-loads
        nc.semaphore("sw_done") as sw_done,  # DVE: w_s half ready (1, 2)
        nc.semaphore("sx_done") as sx_done,  # DVE: x_s ready (count = tiles)
        nc.semaphore("mm_done") as mm_done,  # PE: 3 incs per b-tile
        nc.semaphore("cmb_done") as cmb_done,  # DVE: combines done (per tile)
        nc.semaphore("dma_out") as dma_out,  # ACT ring: output stores
    ):

        @block.sync
        def _(sp):
            for t in range(BT):
                if t >= XD:
                    # buffer slot t%XD free once all 3 groups of t-XD ran
                    sp.wait_ge(mm_done, 3 * (t - XD + 1))
                sp.dma_start(
                    out=x_re_sb.ap()[:, t % XD, :, :],
                    in_=xt_re.ap()[t],
                ).then_inc(dma_x, 16)
                sp.dma_start(
                    out=x_im_sb.ap()[:, t % XD, :, :],
                    in_=xt_im.ap()[t],
                ).then_inc(dma_x, 16)

        @block.scalar
        def _(act):
            # 4 half-tensor loads: re_lo, im_lo, re_hi, im_hi
            act.dma_start(
                out=w_re_sb.ap()[:, 0:KH, :], in_=wt_re.ap()[:, 0:KH, :]
            ).then_inc(dma_w, 16)
            act.dma_start(
                out=w_im_sb.ap()[:, 0:KH, :], in_=wt_im.ap()[:, 0:KH, :]
            ).then_inc(dma_w, 16)
            act.dma_start(
                out=w_re_sb.ap()[:, KH:KT, :], in_=wt_re.ap()[:, KH:KT, :]
            ).then_inc(dma_w, 16)
            act.dma_start(
                out=w_im_sb.ap()[:, KH:KT, :], in_=wt_im.ap()[:, KH:KT, :]
            ).then_inc(dma_w, 16)
            for t in range(BT):
                b = t % 2
                act.wait_ge(cmb_done, t + 1)
                act.dma_start(
                    out=out_d.ap()[0, t * 128:(t + 1) * 128, :],
                    in_=o_re_sb.ap()[:, b, :],
                ).then_inc(dma_out, 16)
                act.dma_start(
                    out=out_d.ap()[1, t * 128:(t + 1) * 128, :],
                    in_=o_im_sb.ap()[:, b, :],
                ).then_inc(dma_out, 16)

        @block.vector
        def _(dve):
            # w_s = w_re + w_im, in the two halves as they land
            dve.wait_ge(dma_w, 32)
            dve.tensor_add(
                w_s_sb.ap()[:, 0:KH, :],
                w_re_sb.ap()[:, 0:KH, :],
                w_im_sb.ap()[:, 0:KH, :],
            ).then_inc(sw_done, 1)
            dve.wait_ge(dma_w, 64)
            dve.tensor_add(
                w_s_sb.ap()[:, KH:KT, :],
                w_re_sb.ap()[:, KH:KT, :],
                w_im_sb.ap()[:, KH:KT, :],
            ).then_inc(sw_done, 1)

            def sx(t):
                dve.wait_ge(dma_x, 32 * (t + 1))
                dve.tensor_add(
                    x_s_sb.ap()[:, t % XD, :, :],
                    x_re_sb.ap()[:, t % XD, :, :],
                    x_im_sb.ap()[:, t % XD, :, :],
                ).then_inc(sx_done, 1)

            def cmb(t):
                # DVE tensor_tensor may read at most one PSUM operand, so
                # stage t_rr into SBUF first.
                b = t % 2
                if t >= 2:
                    # staging buffer reuse: outputs of t-2 flushed
                    dve.wait_ge(dma_out, 32 * (t - 1))
                dve.wait_ge(mm_done, 3 * t + 1)
                dve.tensor_copy(r_sb.ap()[:, b, :], p_rr[b].ap())
                dve.wait_ge(mm_done, 3 * t + 2)
                dve.tensor_sub(
                    o_re_sb.ap()[:, b, :], r_sb.ap()[:, b, :], p_ii[b].ap()
                )
                dve.wait_ge(mm_done, 3 * t + 3)
                dve.tensor_sub(
                    o_im_sb.ap()[:, b, :], p_ss[b].ap(), r_sb.ap()[:, b, :]
                )
                dve.tensor_sub(
                    o_im_sb.ap()[:, b, :], o_im_sb.ap()[:, b, :], p_ii[b].ap()
                ).then_inc(cmb_done, 1)

            for t in range(min(XD, BT)):
                sx(t)
            for t in range(BT):
                cmb(t)
                if t + XD < BT:
                    sx(t + XD)

        @block.tensor
        def _(pe):
            for t in range(BT):
                b = t % 2
                groups = (
                    (x_re_sb, w_re_sb, p_rr[b], (16, 48)),
                    (x_im_sb, w_im_sb, p_ii[b], (32, 64)),
                    (x_s_sb, w_s_sb, p_ss[b], None),
                )
                pe.wait_ge(dma_x, 32 * t + 16)
                if t >= 2:
                    pe.wait_ge(cmb_done, t - 1)
                for gi, (xs, ws, ps, wthr) in enumerate(groups):
                    if gi == 1:
                        pe.wait_ge(dma_x, 32 * (t + 1))
                    elif gi == 2:
                        pe.wait_ge(sx_done, t + 1)
                    for k in range(KT):
                        # weight gating for the first two tiles only; after
                        # that program order guarantees residency.
                        if t <= 1 and k in (0, KH):
                            if wthr is not None:
                                pe.wait_ge(dma_w, wthr[0 if k == 0 else 1])
                            else:
                                pe.wait_ge(sw_done, 1 if k == 0 else 2)
                        mm = pe.matmul(
                            out=ps.ap(),
                            lhsT=xs.ap()[:, t % XD, k, :],
                            rhs=ws.ap()[:, k, :],
                            start=(k == 0),
                            stop=(k == KT - 1),
                        )
                    mm.then_inc(mm_done, 1)

    return nc


_NC = None
LAST_RES = None  # last BassKernelResults (for test harness introspection)


def _get_nc() -> bass.Bass:
    global _NC
    if _NC is None:
        _NC = build_nc()
    return _NC


def _pack_x(x_half: np.ndarray) -> np.ndarray:
    # [4096, 2048] -> [BT, 128p, KT, 128b]; element [t,p,k,b] = x[t*128+b, k*128+p]
    return np.ascontiguousarray(
        x_half.reshape(BT, 128, KT, 128).transpose(0, 3, 2, 1)
    )


def _pack_w(w_q: np.ndarray) -> np.ndarray:
    # [512, 2048] (out, in) -> [128p, KT, 512o]; element [p,k,o] = w[o, k*128+p]
    return np.ascontiguousarray(
        w_q.T.reshape(KT, 128, O_SH).transpose(1, 0, 2)
    )


def kernel(x_re, x_im, w_re, w_im):
    x_re = np.asarray(x_re, dtype=np.float32)
    x_im = np.asarray(x_im, dtype=np.float32)
    w_re = np.asarray(w_re, dtype=np.float32)
    w_im = np.asarray(w_im, dtype=np.float32)

    xt_re_h = [_pack_x(x_re[h * B_SH:(h + 1) * B_SH]) for h in range(B_SHARDS)]
    xt_im_h = [_pack_x(x_im[h * B_SH:(h + 1) * B_SH]) for h in range(B_SHARDS)]
    wt_re_q = [_pack_w(w_re[q * O_SH:(q + 1) * O_SH]) for q in range(O_SHARDS)]
    wt_im_q = [_pack_w(w_im[q * O_SH:(q + 1) * O_SH]) for q in range(O_SHARDS)]

    in_maps = []
    for c in range(N_CORES):
        bs, os_ = c // O_SHARDS, c % O_SHARDS
        in_maps.append(
            {
                "xt_re": xt_re_h[bs],
                "xt_im": xt_im_h[bs],
                "wt_re": wt_re_q[os_],
                "wt_im": wt_im_q[os_],
            }
        )

    nc = _get_nc()
    res = run_bass_kernel_spmd(nc, in_maps, core_ids=list(range(N_CORES)))
    global LAST_RES
    LAST_RES = res

    out = np.empty((2, BATCH, OUT_F), dtype=np.float32)
    for c in range(N_CORES):
        bs, os_ = c // O_SHARDS, c % O_SHARDS
        out[:, bs * B_SH:(bs + 1) * B_SH, os_ * O_SH:(os_ + 1) * O_SH] = (
            res.results[c]["out"]
        )
    return out
